# revision 7
# baseline (speedup 1.0000x reference)
"""Trainium2 Bass kernel for 8-head MultiHeadAttention (B=2, S=4096, E=512).

Sharding: 8 cores = 2 batches x 4 query-row chunks of 1024; each core computes
all 8 heads for its (batch, q-range), so there are no collectives.

Key structure (v2, fp8 DoubleRow):
- scores = (Wq xq)^T (Wk xk) = (M^T xq)^T xk with M = Wq^T Wk folded on host,
  so the key side needs NO on-device projection: raw key columns are shipped
  as fp8 in DoubleRow [32,2,*] layout and only the query side is projected
  (bf16 matmul with M, rounded to fp8).
- QK^T and attn@V run as fp8e4 DoubleRow matmuls (0.5 cycles/row, 2x bf16).
- The mask is applied by the PE: a zero-padded-identity DoubleRow matmul adds
  mb8 (+22 unmasked / -240 masked) into the score PSUM before exp, which both
  masks (exp -> 0) and biases scores into the fp8-exp-safe window.
- exp is split across two engines: ACT tiles use activation(Exp)->fp8 out;
  DVE tiles use a Schraudolph bit-trick: round((s+22)*log2e*8) = the bit
  pattern of e4m3(exp(s/8)*0.1222), computed by one tensor_scalar
  (mult 1.4427, max 0.0) into an int8 view of the fp8 pt tile. max-0 clamps
  masked and tiny weights to +0.0. Softmax normalization cancels the 0.1222.
- V is unprojected (+ a ones column for the denominator); Wv is applied after
  normalization on the small [64, q] normalized context (algebraic fusion,
  same as baseline).
"""
import sys
for _p in ('/root/.axon_site/_ro/trn_rl_repo', '/opt/trn_rl_repo'):
    if _p not in sys.path:
        sys.path.append(_p)

import numpy as np
import ml_dtypes

import concourse.bass as bass
import concourse.tile as tile
from concourse import bacc, mybir
from concourse import bass_utils

F32 = mybir.dt.float32
BF16 = mybir.dt.bfloat16
FP8 = mybir.dt.float8e4
I8 = mybir.dt.int8
AF = mybir.ActivationFunctionType
ALU = mybir.AluOpType
DR = mybir.MatmulPerfMode.DoubleRow

N_CORES = 8
B, S, E, H, DH = 2, 4096, 512, 8, 64
QLEN = S // 4           # 1024 q rows per core
KC = S // 128           # 32 k chunks
VBLK = 8 * 66           # 528: per-kc block in valp8 (66 cols/head, 16B-align)

# Schraudolph constants: pt = exp(s/8)*C with C = e^(22/8)*2^-7 = 0.122173
LOG2E_8 = 1.4426950408889634   # d(bits)/d(s) = 8*log2(e)/8
ACT_BIAS = -4.852365           # exp(0.125*(s+22) + bias) = exp(s/8)*C
MB_ON, MB_OFF = 22.0, -240.0

# tile class: DVE when (t*7)%15 < 7 (t = 2*kc+qw) -> near-alternating
# A/D pattern with counts matched to engine rates (1038 vs 1192 ns/tile)
def _is_dve(t):
    return (t * 7) % 15 < 7

_CACHE = {}


def _build_module():
    nc = bacc.Bacc("TRN2", target_bir_lowering=False, debug=False,
                   enable_asserts=True, num_devices=N_CORES)

    xqT = nc.dram_tensor("xqT", [E, QLEN], BF16, kind="ExternalInput").ap()
    xk8d = nc.dram_tensor("xk8d", [4, 128, 2 * S], FP8, kind="ExternalInput").ap()
    valp8d = nc.dram_tensor("valp8d", [128, KC * VBLK], FP8, kind="ExternalInput").ap()
    mb8d = nc.dram_tensor("mb8d", [128, KC * QLEN], FP8, kind="ExternalInput").ap()
    id8d = nc.dram_tensor("id8d", [128, 256], FP8, kind="ExternalInput").ap()
    wmd = nc.dram_tensor("wmd", [128, DH], BF16, kind="ExternalInput").ap()
    wvT = nc.dram_tensor("wvT", [DH, DH], BF16, kind="ExternalInput").ap()
    woT = nc.dram_tensor("woT", [E, E], BF16, kind="ExternalInput").ap()
    bo_b = nc.dram_tensor("bo_b", [128, E], F32, kind="ExternalInput").ap()
    out = nc.dram_tensor("out", [QLEN, E], F32, kind="ExternalOutput").ap()

    with tile.TileContext(nc) as tc:
        _emit(tc, nc, xqT, xk8d, valp8d, mb8d, id8d, wmd, wvT, woT, bo_b, out)

    nc.compile()
    return nc


def _emit(tc, nc, xqT, xk8d, valp8d, mb8d, id8d, wmd, wvT, woT, bo_b, out):
    from contextlib import ExitStack
    ctx = ExitStack()
    const = ctx.enter_context(tc.tile_pool(name="const", bufs=1))
    qpool = ctx.enter_context(tc.tile_pool(name="qp8", bufs=1))
    xqpool = ctx.enter_context(tc.tile_pool(name="xq", bufs=1))
    ptpool = ctx.enter_context(tc.tile_pool(name="pt", bufs=6))
    usbp = ctx.enter_context(tc.tile_pool(name="usb", bufs=8))
    rcpool = ctx.enter_context(tc.tile_pool(name="rc", bufs=4))
    ospool = ctx.enter_context(tc.tile_pool(name="osb", bufs=2))
    psp = ctx.enter_context(tc.tile_pool(name="psp", bufs=3, space="PSUM"))
    uacc = ctx.enter_context(tc.tile_pool(name="uacc", bufs=2, space="PSUM"))

    # ---- small constants (critical path first) ----
    wm_sb = const.tile([128, DH], BF16, tag="wm")   # M at rows 0:64 and 64:128
    nc.sync.dma_start(wm_sb, wmd)
    id8 = const.tile([128, 256], FP8, tag="id8")    # [I128 | zeros]
    nc.sync.dma_start(id8, id8d)
    bias_t = const.tile([128, 1], F32, tag="biast")
    nc.vector.memset(bias_t, ACT_BIAS)
    ones_sb = const.tile([65, DH], BF16, tag="ones")
    nc.vector.memset(ones_sb, 1.0)

    xq_sb = []   # per pair: [128, QLEN] bf16, head-even rows 0:64, odd 64:128
    for pair in range(4):
        t = xqpool.tile([128, QLEN], BF16, tag=f"xq{pair}", name=f"xq{pair}")
        xq_sb.append(t)

    def load_xq(pair, eng):
        eng.dma_start(xq_sb[pair][0:DH, :],
                      xqT[(2 * pair) * DH:(2 * pair + 1) * DH, :])
        eng.dma_start(xq_sb[pair][DH:128, :],
                      xqT[(2 * pair + 1) * DH:(2 * pair + 2) * DH, :])

    load_xq(0, nc.sync)

    xk8 = []     # per pair: [128, 2*S] fp8, head-even parts 0:32, odd 64:96
    for pair in range(4):
        xk8.append(const.tile([128, 2 * S], FP8, tag=f"xk8{pair}",
                              name=f"xk8{pair}"))

    def load_xk8(pair, eng):
        eng.dma_start(xk8[pair][0:32, :], xk8d[pair, 0:32, :])
        eng.dma_start(xk8[pair][64:96, :], xk8d[pair, 64:96, :])

    valp8 = const.tile([128, KC * VBLK], FP8, tag="valp8")
    mb8 = const.tile([128, KC * QLEN], FP8, tag="mb8")
    wv_sb = const.tile([DH, DH], BF16, tag="wv")
    wo_sb = [const.tile([128, E], BF16, tag=f"wo{pc}", name=f"wo{pc}")
             for pc in range(4)]
    bo_sb = const.tile([128, E], F32, tag="bo")

    # staged bulk loads, all on the otherwise-idle SP queue in priority
    # order: first-kc slices first so attention can start early. Pool is
    # kept free for the qp8/usb/norm copies.
    load_xk8(0, nc.sync)
    nc.sync.dma_start(mb8[:, 0:8 * QLEN], mb8d[:, 0:8 * QLEN])
    nc.sync.dma_start(valp8[:, 0:8 * VBLK], valp8d[:, 0:8 * VBLK])

    def load_bulk():
        nc.sync.dma_start(mb8[:, 8 * QLEN:16 * QLEN], mb8d[:, 8 * QLEN:16 * QLEN])
        nc.sync.dma_start(valp8[:, 8 * VBLK:16 * VBLK], valp8d[:, 8 * VBLK:16 * VBLK])
        nc.sync.dma_start(mb8[:, 16 * QLEN:24 * QLEN], mb8d[:, 16 * QLEN:24 * QLEN])
        nc.sync.dma_start(mb8[:, 24 * QLEN:32 * QLEN], mb8d[:, 24 * QLEN:32 * QLEN])
        nc.sync.dma_start(valp8[:, 16 * VBLK:24 * VBLK], valp8d[:, 16 * VBLK:24 * VBLK])
        nc.sync.dma_start(valp8[:, 24 * VBLK:32 * VBLK], valp8d[:, 24 * VBLK:32 * VBLK])
        load_xq(1, nc.sync)
        load_xk8(1, nc.sync)
        load_xq(2, nc.sync)
        load_xk8(2, nc.sync)
        load_xq(3, nc.sync)
        load_xk8(3, nc.sync)
        nc.sync.dma_start(wv_sb, wvT)
        for pc in range(4):
            nc.sync.dma_start(wo_sb[pc], woT[pc * 128:(pc + 1) * 128, :])
        nc.sync.dma_start(bo_sb, bo_b)

    qp8 = [qpool.tile([128, 2 * QLEN], FP8, tag=f"qp8{pair}", name=f"qp8{pair}")
           for pair in range(4)]

    # ---- q-side projection: qp8[pair] = fp8(M^T xq) in DR [32,2,512] layout
    def qproj_unit(pair, qc):
        def go():
            ut = psp.tile([128, 1024], F32, tag="ps", name=f"qp{pair}_{qc}")
            for h2 in range(2):
                for i in range(2):
                    nc.tensor.matmul(
                        ut[h2 * 64:h2 * 64 + 32, i * 512:(i + 1) * 512],
                        lhsT=wm_sb[h2 * 64:(h2 + 1) * 64, i * 32:(i + 1) * 32],
                        rhs=xq_sb[pair][h2 * 64:(h2 + 1) * 64,
                                        qc * 512:(qc + 1) * 512],
                        start=True, stop=True)
            dst = qp8[pair][:, qc * 1024:(qc + 1) * 1024]
            dv = bass.AP(tensor=dst.tensor, offset=dst.offset,
                         ap=[dst.ap[0], [512, 2], [1, 512]])
            sv = bass.AP(tensor=ut.tensor, offset=ut.offset,
                         ap=[ut.ap[0], [512, 2], [1, 512]])
            nc.gpsimd.tensor_copy(dv, sv)
        return go

    def xk8_ap(pair, h2, kc):
        t = xk8[pair][h2 * 64:h2 * 64 + 32, :]
        return bass.AP(tensor=t.tensor, offset=t.offset + kc * 128,
                       ap=[t.ap[0], [S, 2], [1, 128]])

    def qp8_ap(pair, h2, qw):
        t = qp8[pair][h2 * 64:h2 * 64 + 32, :]
        return bass.AP(tensor=t.tensor, offset=t.offset + qw * 1024,
                       ap=[t.ap[0], [512, 2], [1, 512]])

    def mb8_ap(kc, qw):
        return bass.AP(tensor=mb8.tensor, offset=mb8.offset + kc * QLEN + qw * 512,
                       ap=[mb8.ap[0], [0, 2], [1, 512]])

    def id8_ap():
        return bass.AP(tensor=id8.tensor, offset=id8.offset,
                       ap=[id8.ap[0], [128, 2], [1, 128]])

    def valp8_ap(j, h):
        return bass.AP(tensor=valp8.tensor,
                       offset=valp8.offset + (2 * j) * VBLK + h * 66,
                       ap=[valp8.ap[0], [VBLK, 2], [1, 65]])

    def pt_ap(pt, h2):
        return bass.AP(tensor=pt.tensor, offset=pt.offset + h2 * 512,
                       ap=[pt.ap[0], [1024, 2], [1, 512]])

    concatT = [const.tile([128, QLEN], BF16, tag=f"ct{pair}", name=f"ct{pair}")
               for pair in range(4)]

    def attn(pair, qw, trickle=()):
        trickle = list(trickle)
        U = [uacc.tile([65, 512], F32, tag="u", name=f"U{pair}_{qw}_{h2}")
             for h2 in range(2)]

        def attnv(j, pt):
            def go():
                for h2 in range(2):
                    nc.tensor.matmul(U[h2], lhsT=valp8_ap(j, 2 * pair + h2),
                                     rhs=pt_ap(pt, h2),
                                     start=(j == 0), stop=(j == KC // 2 - 1),
                                     perf_mode=DR)
            return go

        prev_av = None
        for j in range(KC // 2):
            pt = ptpool.tile([128, 2048], FP8, tag="pt",
                             name=f"pt{pair}_{qw}_{j}")
            pti8 = pt.bitcast(I8)
            for half in range(2):
                kc = 2 * j + half
                ps = psp.tile([128, 1024], F32, tag="ps",
                              name=f"ps{pair}_{qw}_{kc}")
                for h2 in range(2):
                    sl = ps[:, h2 * 512:(h2 + 1) * 512]
                    nc.tensor.matmul(sl, lhsT=xk8_ap(pair, h2, kc),
                                     rhs=qp8_ap(pair, h2, qw),
                                     start=True, stop=False, perf_mode=DR)
                    nc.tensor.matmul(sl, lhsT=id8_ap(), rhs=mb8_ap(kc, qw),
                                     start=False, stop=True, perf_mode=DR)
                if _is_dve(2 * kc + qw):
                    nc.vector.tensor_scalar(
                        pti8[:, half * 1024:(half + 1) * 1024], ps,
                        LOG2E_8, 0.0, ALU.mult, ALU.max)
                else:
                    nc.scalar.activation(pt[:, half * 1024:(half + 1) * 1024],
                                         ps, AF.Exp, bias=bias_t, scale=0.125)
            # software pipeline: attn@V one stage late so the in-order PE
            # never makes the exp engines wait.
            if prev_av is not None:
                prev_av()
            prev_av = attnv(j, pt)
            if trickle and (j % 4 == 3):
                trickle.pop(0)()
        prev_av()
        for work in trickle:
            work()
        usb = [usbp.tile([65, 512], F32, tag="usb", name=f"usb{pair}_{qw}_{h2}")
               for h2 in range(2)]
        for h2 in range(2):
            nc.gpsimd.tensor_copy(usb[h2], U[h2])

        def norm_one(h2, qlo=0, qwidth=512):
            def go():
                u = usb[h2]
                rc = rcpool.tile([65, 512], BF16, tag="rc",
                                 name=f"rc{pair}_{qw}_{h2}_{qlo}")
                with nc.allow_low_precision(reason="1/D broadcast in bf16"):
                    nc.vector.reciprocal(rc[64:65, qlo:qlo + qwidth],
                                         u[64:65, qlo:qlo + qwidth])
                ut = psp.tile([128, 1024], F32, tag="ps",
                              name=f"nm{pair}_{qw}_{h2}_{qlo}")
                rb = ut[0:64, 0:512]
                nc.tensor.matmul(rb[:, 0:qwidth], lhsT=ones_sb[64:65, :],
                                 rhs=rc[64:65, qlo:qlo + qwidth],
                                 start=True, stop=True)
                un = rcpool.tile([64, 512], BF16, tag="un",
                                 name=f"un{pair}_{qw}_{h2}_{qlo}")
                nc.gpsimd.scalar_tensor_tensor(
                    un[:, 0:qwidth], u[0:64, qlo:qlo + qwidth], 1.0,
                    rb[:, 0:qwidth], ALU.mult, ALU.mult)
                cx = ut[:, 512:1024]
                nc.tensor.matmul(cx[h2 * 64:(h2 + 1) * 64, 0:qwidth],
                                 lhsT=wv_sb, rhs=un[:, 0:qwidth],
                                 start=True, stop=True)
                nc.gpsimd.tensor_copy(
                    concatT[pair][h2 * 64:(h2 + 1) * 64,
                                  qw * 512 + qlo:qw * 512 + qlo + qwidth],
                    cx[h2 * 64:(h2 + 1) * 64, 0:qwidth])
            return go

        return [norm_one(0), norm_one(1)]

    def outproj(qts):
        def one(qt):
            def go():
                ut = psp.tile([128, 1024], F32, tag="ps", name=f"op{qt}")
                op = ut[:, 0:512]
                for pc in range(4):
                    nc.tensor.matmul(op,
                                     lhsT=concatT[pc][:, qt * 128:(qt + 1) * 128],
                                     rhs=wo_sb[pc],
                                     start=(pc == 0), stop=(pc == 3))
                osb = ospool.tile([128, E], F32, tag="osb", name=f"osb{qt}")
                nc.gpsimd.scalar_tensor_tensor(osb, op, 1.0, bo_sb,
                                               ALU.mult, ALU.add)
                nc.sync.dma_start(out[qt * 128:(qt + 1) * 128, :], osb)
            return go
        return [one(qt) for qt in qts]

    # ---- emission schedule ----
    qproj_unit(0, 0)()
    qproj_unit(0, 1)()
    load_bulk()
    n00 = attn(0, 0, trickle=[qproj_unit(1, 0), qproj_unit(1, 1),
                              qproj_unit(2, 0)])
    n01 = attn(0, 1, trickle=n00 + [qproj_unit(2, 1)])
    n10 = attn(1, 0, trickle=n01 + [qproj_unit(3, 0)])
    n11 = attn(1, 1, trickle=n10 + [qproj_unit(3, 1)])
    n20 = attn(2, 0, trickle=n11)
    n21 = attn(2, 1, trickle=n20)
    n30 = attn(3, 0, trickle=n21)
    # last call: trickle pair-3-qw0 norms early, then outproj for q rows
    # that only need qw0 columns (qt 0..3) late in the loop
    op03 = outproj(range(4))
    n31 = attn(3, 1, trickle=n30 + op03)
    # tail
    n31[0]()
    n31[1]()
    for q in outproj(range(4, 8)):
        q()

    ctx.close()


def _prep_inputs(key, query, value, mask, Wq, Wk, Wv, Wo, bo):
    bf16 = ml_dtypes.bfloat16
    e4m3 = ml_dtypes.float8_e4m3
    key = np.asarray(key, np.float32)
    query = np.asarray(query, np.float32)
    value = np.asarray(value, np.float32)
    mask = np.asarray(mask)
    Wq = np.asarray(Wq, np.float32)
    Wk = np.asarray(Wk, np.float32)

    # M = Wq^T Wk so that scores = (xq M) xk^T ; duplicated on rows 64:128
    M = (Wq.T @ Wk).astype(np.float32)
    wm = np.concatenate([M, M], axis=0)  # [128, 64]

    id8 = np.zeros((128, 256), np.float32)
    id8[:, :128] = np.eye(128)

    common = {
        "wmd": wm.astype(bf16),
        "id8d": id8.astype(e4m3),
        "wvT": np.ascontiguousarray(np.asarray(Wv, np.float32).T).astype(bf16),
        "woT": np.ascontiguousarray(np.asarray(Wo, np.float32).T).astype(bf16),
        "bo_b": np.ascontiguousarray(
            np.broadcast_to(np.asarray(bo, np.float32), (128, E))),
    }

    per_b = {}
    for b in range(B):
        # xk8: [4 pairs, 128, 2*S]: head-even parts 0:32, head-odd 64:96
        kT = key[b].T.reshape(H, 2, 32, S)   # [h, i, p, kpos]
        xk8 = np.zeros((4, 128, 2 * S), np.float32)
        for pair in range(4):
            for h2 in range(2):
                xk8[pair, h2 * 64:h2 * 64 + 32, :] = \
                    kT[2 * pair + h2].transpose(1, 0, 2).reshape(32, 2 * S)
        # valp8: [128, KC*528]: (p, kc*528 + h*66 + c), c=64 -> 1.0
        v4 = value[b].reshape(KC, 128, H, DH).transpose(1, 0, 2, 3)
        vb = np.zeros((128, KC, H, 66), np.float32)
        vb[:, :, :, :DH] = v4
        vb[:, :, :, DH] = 1.0
        per_b[b] = {
            "xk8": np.ascontiguousarray(xk8).astype(e4m3),
            "valp8": np.ascontiguousarray(vb.reshape(128, KC * VBLK)).astype(e4m3),
            "qT": query[b].T,
        }

    # mask bias [k, q] -> mb8[p, kc*QLEN + q] per core (q-sliced)
    mT = (mask[0, 0] != 0).T  # [k, q]
    mb_full = np.where(mT, MB_ON, MB_OFF).astype(np.float32)

    in_maps = []
    for c in range(N_CORES):
        b, qs = c // 4, (c % 4) * QLEN
        mb = mb_full[:, qs:qs + QLEN].reshape(KC, 128, QLEN).transpose(1, 0, 2)
        in_maps.append({
            "xqT": np.ascontiguousarray(
                per_b[b]["qT"][:, qs:qs + QLEN]).astype(bf16),
            "xk8d": per_b[b]["xk8"],
            "valp8d": per_b[b]["valp8"],
            "mb8d": np.ascontiguousarray(mb.reshape(128, KC * QLEN)).astype(e4m3),
            **common,
        })
    return in_maps


def get_module():
    if "nc" not in _CACHE:
        _CACHE["nc"] = _build_module()
    return _CACHE["nc"]


def kernel(key, query, value, mask, Wq, Wk, Wv, Wo, bo, **_):
    nc = get_module()
    in_maps = _prep_inputs(key, query, value, mask, Wq, Wk, Wv, Wo, bo)
    res = bass_utils.run_bass_kernel_spmd(
        nc, in_maps, core_ids=list(range(N_CORES)))
    full = np.empty((B, S, E), np.float32)
    for c in range(N_CORES):
        b, qs = c // 4, (c % 4) * QLEN
        full[b, qs:qs + QLEN, :] = res.results[c]["out"]
    return full


# revision 8
# speedup vs baseline: 1.3012x; 1.3012x over previous
"""Trainium2 Bass kernel for 8-head MultiHeadAttention (B=2, S=4096, E=512).

Sharding: 8 cores = 2 batches x 4 query-row chunks of 1024; each core computes
all 8 heads for its (batch, q-range), so there are no collectives.

Key structure (v2, fp8 DoubleRow):
- scores = (Wq xq)^T (Wk xk) = (M^T xq)^T xk with M = Wq^T Wk folded on host,
  so the key side needs NO on-device projection: raw key columns are shipped
  as fp8 in DoubleRow [32,2,*] layout and only the query side is projected
  (bf16 matmul with M, rounded to fp8).
- QK^T and attn@V run as fp8e4 DoubleRow matmuls (0.5 cycles/row, 2x bf16).
- The mask is applied by the PE: a zero-padded-identity DoubleRow matmul adds
  mb8 (+22 unmasked / -240 masked) into the score PSUM before exp, which both
  masks (exp -> 0) and biases scores into the fp8-exp-safe window.
- exp is split across two engines: ACT tiles use activation(Exp)->fp8 out;
  DVE tiles use a Schraudolph bit-trick: round((s+22)*log2e*8) = the bit
  pattern of e4m3(exp(s/8)*0.1222), computed by one tensor_scalar
  (mult 1.4427, max 0.0) into an int8 view of the fp8 pt tile. max-0 clamps
  masked and tiny weights to +0.0. Softmax normalization cancels the 0.1222.
- V is unprojected (+ a ones column for the denominator); Wv is applied after
  normalization on the small [64, q] normalized context (algebraic fusion,
  same as baseline).
"""
import sys
for _p in ('/root/.axon_site/_ro/trn_rl_repo', '/opt/trn_rl_repo'):
    if _p not in sys.path:
        sys.path.append(_p)

import numpy as np
import ml_dtypes

import concourse.bass as bass
import concourse.tile as tile
from concourse import bacc, mybir
from concourse import bass_utils

F32 = mybir.dt.float32
BF16 = mybir.dt.bfloat16
FP8 = mybir.dt.float8e4
I8 = mybir.dt.int8
AF = mybir.ActivationFunctionType
ALU = mybir.AluOpType
DR = mybir.MatmulPerfMode.DoubleRow

N_CORES = 8
B, S, E, H, DH = 2, 4096, 512, 8, 64
QLEN = S // 4           # 1024 q rows per core
KC = S // 128           # 32 k chunks
VBLK = 8 * 66           # 528: per-kc block in valp8 (66 cols/head, 16B-align)

# Schraudolph constants: pt = exp(s/8)*C with C = e^(22/8)*2^-7 = 0.122173
LOG2E_8 = 1.4426950408889634   # d(bits)/d(s) = 8*log2(e)/8
ACT_BIAS = -4.852365           # exp(0.125*(s+22) + bias) = exp(s/8)*C
MB_ON, MB_OFF = 22.0, -240.0

# per-call engine pattern: 8 ACT : 7 DVE per 15 kcs, max same-engine run 2,
# counts matched to per-tile engine rates (1038 vs 1192 ns)
_PAT = "ADADADAADADADAD"


def _is_dve(kc, qw):
    return _PAT[(kc + 7 * qw) % 15] == "D"

_CACHE = {}


def _build_module():
    nc = bacc.Bacc("TRN2", target_bir_lowering=False, debug=False,
                   enable_asserts=True, num_devices=N_CORES)

    xqT = nc.dram_tensor("xqT", [E, QLEN], BF16, kind="ExternalInput").ap()
    xk8d = nc.dram_tensor("xk8d", [4, 128, 2 * S], FP8, kind="ExternalInput").ap()
    valp8d = nc.dram_tensor("valp8d", [128, KC * VBLK], FP8, kind="ExternalInput").ap()
    mb8d = nc.dram_tensor("mb8d", [128, KC * QLEN], FP8, kind="ExternalInput").ap()
    id8d = nc.dram_tensor("id8d", [128, 256], FP8, kind="ExternalInput").ap()
    wmd = nc.dram_tensor("wmd", [128, DH], BF16, kind="ExternalInput").ap()
    wvT = nc.dram_tensor("wvT", [DH, DH], BF16, kind="ExternalInput").ap()
    woT = nc.dram_tensor("woT", [E, E], BF16, kind="ExternalInput").ap()
    bo_b = nc.dram_tensor("bo_b", [128, E], F32, kind="ExternalInput").ap()
    out = nc.dram_tensor("out", [QLEN, E], F32, kind="ExternalOutput").ap()

    with tile.TileContext(nc) as tc:
        _emit(tc, nc, xqT, xk8d, valp8d, mb8d, id8d, wmd, wvT, woT, bo_b, out)

    nc.compile()
    return nc


def _emit(tc, nc, xqT, xk8d, valp8d, mb8d, id8d, wmd, wvT, woT, bo_b, out):
    from contextlib import ExitStack
    ctx = ExitStack()
    const = ctx.enter_context(tc.tile_pool(name="const", bufs=1))
    qpool = ctx.enter_context(tc.tile_pool(name="qp8", bufs=1))
    xqpool = ctx.enter_context(tc.tile_pool(name="xq", bufs=1))
    ptpool = ctx.enter_context(tc.tile_pool(name="pt", bufs=6))
    usbp = ctx.enter_context(tc.tile_pool(name="usb", bufs=8))
    rcpool = ctx.enter_context(tc.tile_pool(name="rc", bufs=4))
    ospool = ctx.enter_context(tc.tile_pool(name="osb", bufs=2))
    psp = ctx.enter_context(tc.tile_pool(name="psp", bufs=3, space="PSUM"))
    uacc = ctx.enter_context(tc.tile_pool(name="uacc", bufs=2, space="PSUM"))

    # ---- small constants (critical path first) ----
    wm_sb = const.tile([128, DH], BF16, tag="wm")   # M at rows 0:64 and 64:128
    nc.sync.dma_start(wm_sb, wmd)
    id8 = const.tile([128, 256], FP8, tag="id8")    # [I128 | zeros]
    nc.sync.dma_start(id8, id8d)
    bias_t = const.tile([128, 1], F32, tag="biast")
    nc.vector.memset(bias_t, ACT_BIAS)
    ones_sb = const.tile([65, DH], BF16, tag="ones")
    nc.vector.memset(ones_sb, 1.0)
    ones_row = const.tile([1, 512], F32, tag="onesr")
    nc.vector.memset(ones_row, 1.0)

    xq_sb = []   # per pair: [128, QLEN] bf16, head-even rows 0:64, odd 64:128
    for pair in range(4):
        t = xqpool.tile([128, QLEN], BF16, tag=f"xq{pair}", name=f"xq{pair}")
        xq_sb.append(t)

    def load_xq(pair, eng):
        eng.dma_start(xq_sb[pair][0:DH, :],
                      xqT[(2 * pair) * DH:(2 * pair + 1) * DH, :])
        eng.dma_start(xq_sb[pair][DH:128, :],
                      xqT[(2 * pair + 1) * DH:(2 * pair + 2) * DH, :])

    load_xq(0, nc.sync)

    xk8 = []     # per pair: [128, 2*S] fp8, head-even parts 0:32, odd 64:96
    for pair in range(4):
        xk8.append(const.tile([128, 2 * S], FP8, tag=f"xk8{pair}",
                              name=f"xk8{pair}"))

    def load_xk8(pair, eng):
        eng.dma_start(xk8[pair][0:32, :], xk8d[pair, 0:32, :])
        eng.dma_start(xk8[pair][64:96, :], xk8d[pair, 64:96, :])

    valp8 = const.tile([128, KC * VBLK], FP8, tag="valp8")
    mb8 = const.tile([128, KC * QLEN], FP8, tag="mb8")
    wv_sb = const.tile([DH, DH], BF16, tag="wv")
    wo_sb = [const.tile([128, E], BF16, tag=f"wo{pc}", name=f"wo{pc}")
             for pc in range(4)]
    bo_sb = const.tile([128, E], F32, tag="bo")

    # staged bulk loads, all on the otherwise-idle SP queue in priority
    # order: first-kc slices first so attention can start early. Pool is
    # kept free for the qp8/usb/norm copies.
    load_xk8(0, nc.sync)
    nc.sync.dma_start(mb8[:, 0:8 * QLEN], mb8d[:, 0:8 * QLEN])
    nc.sync.dma_start(valp8[:, 0:8 * VBLK], valp8d[:, 0:8 * VBLK])

    def load_bulk():
        nc.sync.dma_start(mb8[:, 8 * QLEN:16 * QLEN], mb8d[:, 8 * QLEN:16 * QLEN])
        nc.sync.dma_start(valp8[:, 8 * VBLK:16 * VBLK], valp8d[:, 8 * VBLK:16 * VBLK])
        nc.sync.dma_start(mb8[:, 16 * QLEN:24 * QLEN], mb8d[:, 16 * QLEN:24 * QLEN])
        nc.sync.dma_start(mb8[:, 24 * QLEN:32 * QLEN], mb8d[:, 24 * QLEN:32 * QLEN])
        nc.sync.dma_start(valp8[:, 16 * VBLK:24 * VBLK], valp8d[:, 16 * VBLK:24 * VBLK])
        nc.sync.dma_start(valp8[:, 24 * VBLK:32 * VBLK], valp8d[:, 24 * VBLK:32 * VBLK])
        load_xq(1, nc.sync)
        load_xk8(1, nc.sync)
        load_xq(2, nc.sync)
        load_xk8(2, nc.sync)
        load_xq(3, nc.sync)
        load_xk8(3, nc.sync)
        nc.sync.dma_start(wv_sb, wvT)
        for pc in range(4):
            nc.sync.dma_start(wo_sb[pc], woT[pc * 128:(pc + 1) * 128, :])
        nc.sync.dma_start(bo_sb, bo_b)

    qp8 = [qpool.tile([128, 2 * QLEN], FP8, tag=f"qp8{pair}", name=f"qp8{pair}")
           for pair in range(4)]

    # ---- q-side projection: qp8[pair] = fp8(M^T xq) in DR [32,2,512] layout
    def qproj_unit(pair, qc):
        def go():
            ut = psp.tile([128, 1024], F32, tag="ps", name=f"qp{pair}_{qc}")
            for h2 in range(2):
                for i in range(2):
                    nc.tensor.matmul(
                        ut[h2 * 64:h2 * 64 + 32, i * 512:(i + 1) * 512],
                        lhsT=wm_sb[h2 * 64:(h2 + 1) * 64, i * 32:(i + 1) * 32],
                        rhs=xq_sb[pair][h2 * 64:(h2 + 1) * 64,
                                        qc * 512:(qc + 1) * 512],
                        start=True, stop=True)
            dst = qp8[pair][:, qc * 1024:(qc + 1) * 1024]
            dv = bass.AP(tensor=dst.tensor, offset=dst.offset,
                         ap=[dst.ap[0], [512, 2], [1, 512]])
            sv = bass.AP(tensor=ut.tensor, offset=ut.offset,
                         ap=[ut.ap[0], [512, 2], [1, 512]])
            nc.gpsimd.tensor_copy(dv, sv)
        return go

    def xk8_ap(pair, h2, kc):
        t = xk8[pair][h2 * 64:h2 * 64 + 32, :]
        return bass.AP(tensor=t.tensor, offset=t.offset + kc * 128,
                       ap=[t.ap[0], [S, 2], [1, 128]])

    def qp8_ap(pair, h2, qw):
        t = qp8[pair][h2 * 64:h2 * 64 + 32, :]
        return bass.AP(tensor=t.tensor, offset=t.offset + qw * 1024,
                       ap=[t.ap[0], [512, 2], [1, 512]])

    def mb8_ap(kc, qw):
        return bass.AP(tensor=mb8.tensor, offset=mb8.offset + kc * QLEN + qw * 512,
                       ap=[mb8.ap[0], [0, 2], [1, 512]])

    def id8_ap():
        return bass.AP(tensor=id8.tensor, offset=id8.offset,
                       ap=[id8.ap[0], [128, 2], [1, 128]])

    def valp8_ap(j, h):
        return bass.AP(tensor=valp8.tensor,
                       offset=valp8.offset + (2 * j) * VBLK + h * 66,
                       ap=[valp8.ap[0], [VBLK, 2], [1, 65]])

    def pt_ap(pt, h2):
        return bass.AP(tensor=pt.tensor, offset=pt.offset + h2 * 512,
                       ap=[pt.ap[0], [1024, 2], [1, 512]])

    concatT = [const.tile([128, QLEN], BF16, tag=f"ct{pair}", name=f"ct{pair}")
               for pair in range(4)]

    def attn(pair, qw, trickle=()):
        trickle = list(trickle)
        U = [uacc.tile([65, 512], F32, tag="u", name=f"U{pair}_{qw}_{h2}")
             for h2 in range(2)]

        def attnv(j, pt):
            def go():
                for h2 in range(2):
                    nc.tensor.matmul(U[h2], lhsT=valp8_ap(j, 2 * pair + h2),
                                     rhs=pt_ap(pt, h2),
                                     start=(j == 0), stop=(j == KC // 2 - 1),
                                     perf_mode=DR)
            return go

        prev_av = None
        for j in range(KC // 2):
            pt = ptpool.tile([128, 2048], FP8, tag="pt",
                             name=f"pt{pair}_{qw}_{j}")
            pti8 = pt.bitcast(I8)
            for half in range(2):
                kc = 2 * j + half
                ps = psp.tile([128, 1024], F32, tag="ps",
                              name=f"ps{pair}_{qw}_{kc}")
                for h2 in range(2):
                    sl = ps[:, h2 * 512:(h2 + 1) * 512]
                    nc.tensor.matmul(sl, lhsT=xk8_ap(pair, h2, kc),
                                     rhs=qp8_ap(pair, h2, qw),
                                     start=True, stop=False, perf_mode=DR)
                    nc.tensor.matmul(sl, lhsT=id8_ap(), rhs=mb8_ap(kc, qw),
                                     start=False, stop=True, perf_mode=DR)
                if _is_dve(kc, qw):
                    nc.vector.tensor_scalar(
                        pti8[:, half * 1024:(half + 1) * 1024], ps,
                        LOG2E_8, 0.0, ALU.mult, ALU.max)
                else:
                    nc.scalar.activation(pt[:, half * 1024:(half + 1) * 1024],
                                         ps, AF.Exp, bias=bias_t, scale=0.125)
            # software pipeline: attn@V one stage late so the in-order PE
            # never makes the exp engines wait.
            if prev_av is not None:
                prev_av()
            prev_av = attnv(j, pt)
            if trickle and (j % 4 == 3):
                trickle.pop(0)()
        prev_av()
        for work in trickle:
            work()
        usb = [usbp.tile([65, 512], F32, tag="usb", name=f"usb{pair}_{qw}_{h2}")
               for h2 in range(2)]
        for h2 in range(2):
            nc.gpsimd.tensor_copy(usb[h2], U[h2])

        def norm_one(h2, qlo=0, qwidth=512):
            def go():
                u = usb[h2]
                rc = rcpool.tile([65, 512], BF16, tag="rc",
                                 name=f"rc{pair}_{qw}_{h2}_{qlo}")
                with nc.allow_low_precision(reason="1/D broadcast in bf16"):
                    nc.gpsimd.tensor_tensor(rc[64:65, qlo:qlo + qwidth],
                                            ones_row[:, qlo:qlo + qwidth],
                                            u[64:65, qlo:qlo + qwidth],
                                            ALU.divide)
                ut = psp.tile([128, 1024], F32, tag="ps",
                              name=f"nm{pair}_{qw}_{h2}_{qlo}")
                rb = ut[0:64, 0:512]
                nc.tensor.matmul(rb[:, 0:qwidth], lhsT=ones_sb[64:65, :],
                                 rhs=rc[64:65, qlo:qlo + qwidth],
                                 start=True, stop=True)
                un = rcpool.tile([64, 512], BF16, tag="un",
                                 name=f"un{pair}_{qw}_{h2}_{qlo}")
                nc.gpsimd.scalar_tensor_tensor(
                    un[:, 0:qwidth], u[0:64, qlo:qlo + qwidth], 1.0,
                    rb[:, 0:qwidth], ALU.mult, ALU.mult)
                cx = ut[:, 512:1024]
                nc.tensor.matmul(cx[h2 * 64:(h2 + 1) * 64, 0:qwidth],
                                 lhsT=wv_sb, rhs=un[:, 0:qwidth],
                                 start=True, stop=True)
                nc.gpsimd.tensor_copy(
                    concatT[pair][h2 * 64:(h2 + 1) * 64,
                                  qw * 512 + qlo:qw * 512 + qlo + qwidth],
                    cx[h2 * 64:(h2 + 1) * 64, 0:qwidth])
            return go

        return [norm_one(0), norm_one(1)]

    def outproj(qts):
        def one(qt):
            def go():
                ut = psp.tile([128, 1024], F32, tag="ps", name=f"op{qt}")
                op = ut[:, 0:512]
                for pc in range(4):
                    nc.tensor.matmul(op,
                                     lhsT=concatT[pc][:, qt * 128:(qt + 1) * 128],
                                     rhs=wo_sb[pc],
                                     start=(pc == 0), stop=(pc == 3))
                osb = ospool.tile([128, E], F32, tag="osb", name=f"osb{qt}")
                nc.gpsimd.scalar_tensor_tensor(osb, op, 1.0, bo_sb,
                                               ALU.mult, ALU.add)
                nc.sync.dma_start(out[qt * 128:(qt + 1) * 128, :], osb)
            return go
        return [one(qt) for qt in qts]

    # ---- emission schedule ----
    qproj_unit(0, 0)()
    qproj_unit(0, 1)()
    load_bulk()
    n00 = attn(0, 0, trickle=[qproj_unit(1, 0), qproj_unit(1, 1),
                              qproj_unit(2, 0)])
    n01 = attn(0, 1, trickle=n00 + [qproj_unit(2, 1)])
    n10 = attn(1, 0, trickle=n01 + [qproj_unit(3, 0)])
    n11 = attn(1, 1, trickle=n10 + [qproj_unit(3, 1)])
    n20 = attn(2, 0, trickle=n11)
    n21 = attn(2, 1, trickle=n20)
    n30 = attn(3, 0, trickle=n21)
    # last call: trickle pair-3-qw0 norms early, then outproj for q rows
    # that only need qw0 columns (qt 0..3) late in the loop
    op03 = outproj(range(4))
    n31 = attn(3, 1, trickle=n30 + op03)
    # tail
    n31[0]()
    n31[1]()
    for q in outproj(range(4, 8)):
        q()

    ctx.close()


def _prep_inputs(key, query, value, mask, Wq, Wk, Wv, Wo, bo):
    bf16 = ml_dtypes.bfloat16
    e4m3 = ml_dtypes.float8_e4m3
    key = np.asarray(key, np.float32)
    query = np.asarray(query, np.float32)
    value = np.asarray(value, np.float32)
    mask = np.asarray(mask)
    Wq = np.asarray(Wq, np.float32)
    Wk = np.asarray(Wk, np.float32)

    # M = Wq^T Wk so that scores = (xq M) xk^T ; duplicated on rows 64:128
    M = (Wq.T @ Wk).astype(np.float32)
    wm = np.concatenate([M, M], axis=0)  # [128, 64]

    id8 = np.zeros((128, 256), np.float32)
    id8[:, :128] = np.eye(128)

    common = {
        "wmd": wm.astype(bf16),
        "id8d": id8.astype(e4m3),
        "wvT": np.ascontiguousarray(np.asarray(Wv, np.float32).T).astype(bf16),
        "woT": np.ascontiguousarray(np.asarray(Wo, np.float32).T).astype(bf16),
        "bo_b": np.ascontiguousarray(
            np.broadcast_to(np.asarray(bo, np.float32), (128, E))),
    }

    per_b = {}
    for b in range(B):
        # xk8: [4 pairs, 128, 2*S]: head-even parts 0:32, head-odd 64:96
        kT = key[b].T.reshape(H, 2, 32, S)   # [h, i, p, kpos]
        xk8 = np.zeros((4, 128, 2 * S), np.float32)
        for pair in range(4):
            for h2 in range(2):
                xk8[pair, h2 * 64:h2 * 64 + 32, :] = \
                    kT[2 * pair + h2].transpose(1, 0, 2).reshape(32, 2 * S)
        # valp8: [128, KC*528]: (p, kc*528 + h*66 + c), c=64 -> 1.0
        v4 = value[b].reshape(KC, 128, H, DH).transpose(1, 0, 2, 3)
        vb = np.zeros((128, KC, H, 66), np.float32)
        vb[:, :, :, :DH] = v4
        vb[:, :, :, DH] = 1.0
        per_b[b] = {
            "xk8": np.ascontiguousarray(xk8).astype(e4m3),
            "valp8": np.ascontiguousarray(vb.reshape(128, KC * VBLK)).astype(e4m3),
            "qT": query[b].T,
        }

    # mask bias [k, q] -> mb8[p, kc*QLEN + q] per core (q-sliced)
    mT = (mask[0, 0] != 0).T  # [k, q]
    mb_full = np.where(mT, MB_ON, MB_OFF).astype(np.float32)

    in_maps = []
    for c in range(N_CORES):
        b, qs = c // 4, (c % 4) * QLEN
        mb = mb_full[:, qs:qs + QLEN].reshape(KC, 128, QLEN).transpose(1, 0, 2)
        in_maps.append({
            "xqT": np.ascontiguousarray(
                per_b[b]["qT"][:, qs:qs + QLEN]).astype(bf16),
            "xk8d": per_b[b]["xk8"],
            "valp8d": per_b[b]["valp8"],
            "mb8d": np.ascontiguousarray(mb.reshape(128, KC * QLEN)).astype(e4m3),
            **common,
        })
    return in_maps


def get_module():
    if "nc" not in _CACHE:
        _CACHE["nc"] = _build_module()
    return _CACHE["nc"]


def kernel(key, query, value, mask, Wq, Wk, Wv, Wo, bo, **_):
    nc = get_module()
    in_maps = _prep_inputs(key, query, value, mask, Wq, Wk, Wv, Wo, bo)
    res = bass_utils.run_bass_kernel_spmd(
        nc, in_maps, core_ids=list(range(N_CORES)))
    full = np.empty((B, S, E), np.float32)
    for c in range(N_CORES):
        b, qs = c // 4, (c % 4) * QLEN
        full[b, qs:qs + QLEN, :] = res.results[c]["out"]
    return full


# revision 10
# speedup vs baseline: 1.3196x; 1.0141x over previous
"""Trainium2 Bass kernel for 8-head MultiHeadAttention (B=2, S=4096, E=512).

Sharding: 8 cores = 2 batches x 4 query-row chunks of 1024; each core computes
all 8 heads for its (batch, q-range), so there are no collectives.

Key structure (v2, fp8 DoubleRow):
- scores = (Wq xq)^T (Wk xk) = (M^T xq)^T xk with M = Wq^T Wk folded on host,
  so the key side needs NO on-device projection: raw key columns are shipped
  as fp8 in DoubleRow [32,2,*] layout and only the query side is projected
  (bf16 matmul with M, rounded to fp8).
- QK^T and attn@V run as fp8e4 DoubleRow matmuls (0.5 cycles/row, 2x bf16).
- The mask is applied by the PE: a zero-padded-identity DoubleRow matmul adds
  mb8 (+22 unmasked / -240 masked) into the score PSUM before exp, which both
  masks (exp -> 0) and biases scores into the fp8-exp-safe window.
- exp is split across two engines: ACT tiles use activation(Exp)->fp8 out;
  DVE tiles use a Schraudolph bit-trick: round((s+22)*log2e*8) = the bit
  pattern of e4m3(exp(s/8)*0.1222), computed by one tensor_scalar
  (mult 1.4427, max 0.0) into an int8 view of the fp8 pt tile. max-0 clamps
  masked and tiny weights to +0.0. Softmax normalization cancels the 0.1222.
- V is unprojected (+ a ones column for the denominator); Wv is applied after
  normalization on the small [64, q] normalized context (algebraic fusion,
  same as baseline).
"""
import sys
for _p in ('/root/.axon_site/_ro/trn_rl_repo', '/opt/trn_rl_repo'):
    if _p not in sys.path:
        sys.path.append(_p)

import numpy as np
import ml_dtypes

import concourse.bass as bass
import concourse.tile as tile
from concourse import bacc, mybir
from concourse import bass_utils

F32 = mybir.dt.float32
BF16 = mybir.dt.bfloat16
FP8 = mybir.dt.float8e4
I8 = mybir.dt.int8
AF = mybir.ActivationFunctionType
ALU = mybir.AluOpType
DR = mybir.MatmulPerfMode.DoubleRow

N_CORES = 8
B, S, E, H, DH = 2, 4096, 512, 8, 64
QLEN = S // 4           # 1024 q rows per core
KC = S // 128           # 32 k chunks
VBLK = 8 * 66           # 528: per-kc block in valp8 (66 cols/head, 16B-align)

# Schraudolph constants: pt = exp(s/8)*C with C = e^(22/8)*2^-7 = 0.122173
LOG2E_8 = 1.4426950408889634   # d(bits)/d(s) = 8*log2(e)/8
ACT_BIAS = -4.852365           # exp(0.125*(s+22) + bias) = exp(s/8)*C
MB_ON, MB_OFF = 22.0, -240.0

# per-call engine pattern: 8 ACT : 7 DVE per 15 kcs, max same-engine run 2,
# counts matched to per-tile engine rates (1038 vs 1192 ns)
_PAT = "ADADADAADADADAD"


def _is_dve(kc, qw):
    return _PAT[(kc + 7 * qw) % 15] == "D"

_CACHE = {}


def _build_module():
    nc = bacc.Bacc("TRN2", target_bir_lowering=False, debug=False,
                   enable_asserts=True, num_devices=N_CORES)

    xqT = nc.dram_tensor("xqT", [E, QLEN], BF16, kind="ExternalInput").ap()
    xk8d = nc.dram_tensor("xk8d", [4, 128, 2 * S], FP8, kind="ExternalInput").ap()
    valp8d = nc.dram_tensor("valp8d", [128, KC * VBLK], FP8, kind="ExternalInput").ap()
    mb8d = nc.dram_tensor("mb8d", [128, KC * QLEN], FP8, kind="ExternalInput").ap()
    id8d = nc.dram_tensor("id8d", [128, 256], FP8, kind="ExternalInput").ap()
    wmd = nc.dram_tensor("wmd", [128, DH], BF16, kind="ExternalInput").ap()
    wvT = nc.dram_tensor("wvT", [DH, DH], BF16, kind="ExternalInput").ap()
    woT = nc.dram_tensor("woT", [E, E], BF16, kind="ExternalInput").ap()
    bo16d = nc.dram_tensor("bo16d", [1, E], BF16, kind="ExternalInput").ap()
    out = nc.dram_tensor("out", [QLEN, E], F32, kind="ExternalOutput").ap()

    with tile.TileContext(nc) as tc:
        _emit(tc, nc, xqT, xk8d, valp8d, mb8d, id8d, wmd, wvT, woT, bo16d, out)

    nc.compile()
    return nc


def _emit(tc, nc, xqT, xk8d, valp8d, mb8d, id8d, wmd, wvT, woT, bo16d, out):
    from contextlib import ExitStack
    ctx = ExitStack()
    const = ctx.enter_context(tc.tile_pool(name="const", bufs=1))
    qpool = ctx.enter_context(tc.tile_pool(name="qp8", bufs=1))
    xqpool = ctx.enter_context(tc.tile_pool(name="xq", bufs=1))
    ptpool = ctx.enter_context(tc.tile_pool(name="pt", bufs=6))
    usbp = ctx.enter_context(tc.tile_pool(name="usb", bufs=8))
    rcpool = ctx.enter_context(tc.tile_pool(name="rc", bufs=4))
    ospool = ctx.enter_context(tc.tile_pool(name="osb", bufs=4))
    psp = ctx.enter_context(tc.tile_pool(name="psp", bufs=3, space="PSUM"))
    uacc = ctx.enter_context(tc.tile_pool(name="uacc", bufs=2, space="PSUM"))

    # ---- small constants (critical path first) ----
    wm_sb = const.tile([128, DH], BF16, tag="wm")   # M at rows 0:64 and 64:128
    nc.sync.dma_start(wm_sb, wmd)
    id8 = const.tile([128, 256], FP8, tag="id8")    # [I128 | zeros]
    nc.sync.dma_start(id8, id8d)
    bias_t = const.tile([128, 1], F32, tag="biast")
    nc.vector.memset(bias_t, ACT_BIAS)
    ones_sb = const.tile([65, DH], BF16, tag="ones")
    nc.vector.memset(ones_sb, 1.0)
    ones_row = const.tile([1, 512], F32, tag="onesr")
    nc.vector.memset(ones_row, 1.0)

    xq_sb = []   # per pair: [128, QLEN] bf16, head-even rows 0:64, odd 64:128
    for pair in range(4):
        t = xqpool.tile([128, QLEN], BF16, tag=f"xq{pair}", name=f"xq{pair}")
        xq_sb.append(t)

    def load_xq(pair, eng):
        eng.dma_start(xq_sb[pair][0:DH, :],
                      xqT[(2 * pair) * DH:(2 * pair + 1) * DH, :])
        eng.dma_start(xq_sb[pair][DH:128, :],
                      xqT[(2 * pair + 1) * DH:(2 * pair + 2) * DH, :])

    load_xq(0, nc.sync)

    xk8 = []     # per pair: [128, 2*S] fp8, head-even parts 0:32, odd 64:96
    for pair in range(4):
        xk8.append(const.tile([128, 2 * S], FP8, tag=f"xk8{pair}",
                              name=f"xk8{pair}"))

    def load_xk8(pair, eng):
        eng.dma_start(xk8[pair][0:32, :], xk8d[pair, 0:32, :])
        eng.dma_start(xk8[pair][64:96, :], xk8d[pair, 64:96, :])

    valp8 = const.tile([128, KC * VBLK], FP8, tag="valp8")
    mb8 = const.tile([128, KC * QLEN], FP8, tag="mb8")
    wv_sb = const.tile([DH, DH], BF16, tag="wv")
    wo_sb = [const.tile([128, E], BF16, tag=f"wo{pc}", name=f"wo{pc}")
             for pc in range(4)]
    bo_sb = const.tile([1, E], BF16, tag="bo")
    ones1 = const.tile([1, 128], BF16, tag="ones1")
    nc.vector.memset(ones1, 1.0)

    # staged bulk loads, all on the otherwise-idle SP queue in priority
    # order: first-kc slices first so attention can start early. Pool is
    # kept free for the qp8/usb/norm copies.
    load_xk8(0, nc.sync)
    nc.sync.dma_start(mb8[:, 0:8 * QLEN], mb8d[:, 0:8 * QLEN])
    nc.sync.dma_start(valp8[:, 0:8 * VBLK], valp8d[:, 0:8 * VBLK])

    def load_bulk():
        nc.sync.dma_start(mb8[:, 8 * QLEN:16 * QLEN], mb8d[:, 8 * QLEN:16 * QLEN])
        nc.sync.dma_start(valp8[:, 8 * VBLK:16 * VBLK], valp8d[:, 8 * VBLK:16 * VBLK])
        nc.sync.dma_start(mb8[:, 16 * QLEN:24 * QLEN], mb8d[:, 16 * QLEN:24 * QLEN])
        nc.sync.dma_start(mb8[:, 24 * QLEN:32 * QLEN], mb8d[:, 24 * QLEN:32 * QLEN])
        nc.sync.dma_start(valp8[:, 16 * VBLK:24 * VBLK], valp8d[:, 16 * VBLK:24 * VBLK])
        nc.sync.dma_start(valp8[:, 24 * VBLK:32 * VBLK], valp8d[:, 24 * VBLK:32 * VBLK])
        load_xq(1, nc.sync)
        load_xk8(1, nc.sync)
        load_xq(2, nc.sync)
        load_xk8(2, nc.sync)
        load_xq(3, nc.sync)
        load_xk8(3, nc.sync)
        nc.sync.dma_start(wv_sb, wvT)
        for pc in range(4):
            nc.sync.dma_start(wo_sb[pc], woT[pc * 128:(pc + 1) * 128, :])
        nc.sync.dma_start(bo_sb, bo16d)

    qp8 = [qpool.tile([128, 2 * QLEN], FP8, tag=f"qp8{pair}", name=f"qp8{pair}")
           for pair in range(4)]

    # ---- q-side projection: qp8[pair] = fp8(M^T xq) in DR [32,2,512] layout
    def qproj_unit(pair, qc):
        def go():
            ut = psp.tile([128, 1024], F32, tag="ps", name=f"qp{pair}_{qc}")
            for h2 in range(2):
                for i in range(2):
                    nc.tensor.matmul(
                        ut[h2 * 64:h2 * 64 + 32, i * 512:(i + 1) * 512],
                        lhsT=wm_sb[h2 * 64:(h2 + 1) * 64, i * 32:(i + 1) * 32],
                        rhs=xq_sb[pair][h2 * 64:(h2 + 1) * 64,
                                        qc * 512:(qc + 1) * 512],
                        start=True, stop=True)
            dst = qp8[pair][:, qc * 1024:(qc + 1) * 1024]
            dv = bass.AP(tensor=dst.tensor, offset=dst.offset,
                         ap=[dst.ap[0], [512, 2], [1, 512]])
            sv = bass.AP(tensor=ut.tensor, offset=ut.offset,
                         ap=[ut.ap[0], [512, 2], [1, 512]])
            if (pair + qc) % 2 == 0:
                nc.scalar.copy(dv, sv)
            else:
                nc.vector.tensor_copy(dv, sv)
        return go

    def xk8_ap(pair, h2, kc):
        t = xk8[pair][h2 * 64:h2 * 64 + 32, :]
        return bass.AP(tensor=t.tensor, offset=t.offset + kc * 128,
                       ap=[t.ap[0], [S, 2], [1, 128]])

    def qp8_ap(pair, h2, qw):
        t = qp8[pair][h2 * 64:h2 * 64 + 32, :]
        return bass.AP(tensor=t.tensor, offset=t.offset + qw * 1024,
                       ap=[t.ap[0], [512, 2], [1, 512]])

    def mb8_ap(kc, qw):
        return bass.AP(tensor=mb8.tensor, offset=mb8.offset + kc * QLEN + qw * 512,
                       ap=[mb8.ap[0], [0, 2], [1, 512]])

    def id8_ap():
        return bass.AP(tensor=id8.tensor, offset=id8.offset,
                       ap=[id8.ap[0], [128, 2], [1, 128]])

    def valp8_ap(j, h):
        return bass.AP(tensor=valp8.tensor,
                       offset=valp8.offset + (2 * j) * VBLK + h * 66,
                       ap=[valp8.ap[0], [VBLK, 2], [1, 65]])

    def pt_ap(pt, h2):
        return bass.AP(tensor=pt.tensor, offset=pt.offset + h2 * 512,
                       ap=[pt.ap[0], [1024, 2], [1, 512]])

    concatT = [const.tile([128, QLEN], BF16, tag=f"ct{pair}", name=f"ct{pair}")
               for pair in range(4)]

    def attn(pair, qw, trickle=()):
        trickle = list(trickle)
        U = [uacc.tile([65, 512], F32, tag="u", name=f"U{pair}_{qw}_{h2}")
             for h2 in range(2)]

        def attnv(j, pt):
            def go():
                for h2 in range(2):
                    nc.tensor.matmul(U[h2], lhsT=valp8_ap(j, 2 * pair + h2),
                                     rhs=pt_ap(pt, h2),
                                     start=(j == 0), stop=(j == KC // 2 - 1),
                                     perf_mode=DR)
            return go

        prev_av = None
        for j in range(KC // 2):
            pt = ptpool.tile([128, 2048], FP8, tag="pt",
                             name=f"pt{pair}_{qw}_{j}")
            pti8 = pt.bitcast(I8)
            for half in range(2):
                kc = 2 * j + half
                ps = psp.tile([128, 1024], F32, tag="ps",
                              name=f"ps{pair}_{qw}_{kc}")
                for h2 in range(2):
                    sl = ps[:, h2 * 512:(h2 + 1) * 512]
                    nc.tensor.matmul(sl, lhsT=xk8_ap(pair, h2, kc),
                                     rhs=qp8_ap(pair, h2, qw),
                                     start=True, stop=False, perf_mode=DR)
                    nc.tensor.matmul(sl, lhsT=id8_ap(), rhs=mb8_ap(kc, qw),
                                     start=False, stop=True, perf_mode=DR)
                if _is_dve(kc, qw):
                    nc.vector.tensor_scalar(
                        pti8[:, half * 1024:(half + 1) * 1024], ps,
                        LOG2E_8, 0.0, ALU.mult, ALU.max)
                else:
                    nc.scalar.activation(pt[:, half * 1024:(half + 1) * 1024],
                                         ps, AF.Exp, bias=bias_t, scale=0.125)
            # software pipeline: attn@V one stage late so the in-order PE
            # never makes the exp engines wait.
            if prev_av is not None:
                prev_av()
            prev_av = attnv(j, pt)
            if trickle and (j % 4 == 3):
                trickle.pop(0)()
        prev_av()
        for work in trickle:
            work()
        usb = [usbp.tile([65, 512], F32, tag="usb", name=f"usb{pair}_{qw}_{h2}")
               for h2 in range(2)]
        nc.scalar.copy(usb[0], U[0])
        nc.vector.tensor_copy(usb[1], U[1])

        def norm_one(h2, qlo=0, qwidth=512):
            def go():
                u = usb[h2]
                rc = rcpool.tile([65, 512], BF16, tag="rc",
                                 name=f"rc{pair}_{qw}_{h2}_{qlo}")
                with nc.allow_low_precision(reason="1/D broadcast in bf16"):
                    nc.gpsimd.tensor_tensor(rc[64:65, qlo:qlo + qwidth],
                                            ones_row[:, qlo:qlo + qwidth],
                                            u[64:65, qlo:qlo + qwidth],
                                            ALU.divide)
                ut = psp.tile([128, 1024], F32, tag="ps",
                              name=f"nm{pair}_{qw}_{h2}_{qlo}")
                rb = ut[0:64, 0:512]
                nc.tensor.matmul(rb[:, 0:qwidth], lhsT=ones_sb[64:65, :],
                                 rhs=rc[64:65, qlo:qlo + qwidth],
                                 start=True, stop=True)
                un = rcpool.tile([64, 512], BF16, tag="un",
                                 name=f"un{pair}_{qw}_{h2}_{qlo}")
                nc.vector.scalar_tensor_tensor(
                    un[:, 0:qwidth], u[0:64, qlo:qlo + qwidth], 1.0,
                    rb[:, 0:qwidth], ALU.mult, ALU.mult)
                cx = ut[:, 512:1024]
                nc.tensor.matmul(cx[h2 * 64:(h2 + 1) * 64, 0:qwidth],
                                 lhsT=wv_sb, rhs=un[:, 0:qwidth],
                                 start=True, stop=True)
                nc.scalar.copy(
                    concatT[pair][h2 * 64:(h2 + 1) * 64,
                                  qw * 512 + qlo:qw * 512 + qlo + qwidth],
                    cx[h2 * 64:(h2 + 1) * 64, 0:qwidth])
            return go

        return [norm_one(0), norm_one(1)]

    def outproj(qts):
        def one(qt):
            def go():
                ut = psp.tile([128, 1024], F32, tag="ps", name=f"op{qt}")
                op = ut[:, 0:512]
                for pc in range(4):
                    nc.tensor.matmul(op,
                                     lhsT=concatT[pc][:, qt * 128:(qt + 1) * 128],
                                     rhs=wo_sb[pc],
                                     start=(pc == 0), stop=False)
                nc.tensor.matmul(op, lhsT=ones1, rhs=bo_sb,
                                 start=False, stop=True)
                osb = ospool.tile([128, E], F32, tag="osb", name=f"osb{qt}")
                if qt % 2 == 0:
                    nc.scalar.copy(osb, op)
                else:
                    nc.vector.tensor_copy(osb, op)
                nc.sync.dma_start(out[qt * 128:(qt + 1) * 128, :], osb)
            return go
        return [one(qt) for qt in qts]

    # ---- emission schedule ----
    qproj_unit(0, 0)()
    qproj_unit(0, 1)()
    load_bulk()
    n00 = attn(0, 0, trickle=[qproj_unit(1, 0), qproj_unit(1, 1),
                              qproj_unit(2, 0)])
    n01 = attn(0, 1, trickle=n00 + [qproj_unit(2, 1)])
    n10 = attn(1, 0, trickle=n01 + [qproj_unit(3, 0)])
    n11 = attn(1, 1, trickle=n10 + [qproj_unit(3, 1)])
    n20 = attn(2, 0, trickle=n11)
    n21 = attn(2, 1, trickle=n20)
    n30 = attn(3, 0, trickle=n21)
    # last call: trickle pair-3-qw0 norms early, then outproj for q rows
    # that only need qw0 columns (qt 0..3) late in the loop
    op03 = outproj(range(4))
    n31 = attn(3, 1, trickle=n30 + op03)
    # tail
    n31[0]()
    n31[1]()
    for q in outproj(range(4, 8)):
        q()

    ctx.close()


def _prep_inputs(key, query, value, mask, Wq, Wk, Wv, Wo, bo):
    bf16 = ml_dtypes.bfloat16
    e4m3 = ml_dtypes.float8_e4m3
    key = np.asarray(key, np.float32)
    query = np.asarray(query, np.float32)
    value = np.asarray(value, np.float32)
    mask = np.asarray(mask)
    Wq = np.asarray(Wq, np.float32)
    Wk = np.asarray(Wk, np.float32)

    # M = Wq^T Wk so that scores = (xq M) xk^T ; duplicated on rows 64:128
    M = (Wq.T @ Wk).astype(np.float32)
    wm = np.concatenate([M, M], axis=0)  # [128, 64]

    id8 = np.zeros((128, 256), np.float32)
    id8[:, :128] = np.eye(128)

    common = {
        "wmd": wm.astype(bf16),
        "id8d": id8.astype(e4m3),
        "wvT": np.ascontiguousarray(np.asarray(Wv, np.float32).T).astype(bf16),
        "woT": np.ascontiguousarray(np.asarray(Wo, np.float32).T).astype(bf16),
        "bo16d": np.asarray(bo, np.float32).reshape(1, E).astype(bf16),
    }

    per_b = {}
    for b in range(B):
        # xk8: [4 pairs, 128, 2*S]: head-even parts 0:32, head-odd 64:96
        kT = key[b].T.reshape(H, 2, 32, S)   # [h, i, p, kpos]
        xk8 = np.zeros((4, 128, 2 * S), np.float32)
        for pair in range(4):
            for h2 in range(2):
                xk8[pair, h2 * 64:h2 * 64 + 32, :] = \
                    kT[2 * pair + h2].transpose(1, 0, 2).reshape(32, 2 * S)
        # valp8: [128, KC*528]: (p, kc*528 + h*66 + c), c=64 -> 1.0
        v4 = value[b].reshape(KC, 128, H, DH).transpose(1, 0, 2, 3)
        vb = np.zeros((128, KC, H, 66), np.float32)
        vb[:, :, :, :DH] = v4
        vb[:, :, :, DH] = 1.0
        per_b[b] = {
            "xk8": np.ascontiguousarray(xk8).astype(e4m3),
            "valp8": np.ascontiguousarray(vb.reshape(128, KC * VBLK)).astype(e4m3),
            "qT": query[b].T,
        }

    # mask bias [k, q] -> mb8[p, kc*QLEN + q] per core (q-sliced)
    mT = (mask[0, 0] != 0).T  # [k, q]
    mb_full = np.where(mT, MB_ON, MB_OFF).astype(np.float32)

    in_maps = []
    for c in range(N_CORES):
        b, qs = c // 4, (c % 4) * QLEN
        mb = mb_full[:, qs:qs + QLEN].reshape(KC, 128, QLEN).transpose(1, 0, 2)
        in_maps.append({
            "xqT": np.ascontiguousarray(
                per_b[b]["qT"][:, qs:qs + QLEN]).astype(bf16),
            "xk8d": per_b[b]["xk8"],
            "valp8d": per_b[b]["valp8"],
            "mb8d": np.ascontiguousarray(mb.reshape(128, KC * QLEN)).astype(e4m3),
            **common,
        })
    return in_maps


def get_module():
    if "nc" not in _CACHE:
        _CACHE["nc"] = _build_module()
    return _CACHE["nc"]


def kernel(key, query, value, mask, Wq, Wk, Wv, Wo, bo, **_):
    nc = get_module()
    in_maps = _prep_inputs(key, query, value, mask, Wq, Wk, Wv, Wo, bo)
    res = bass_utils.run_bass_kernel_spmd(
        nc, in_maps, core_ids=list(range(N_CORES)))
    full = np.empty((B, S, E), np.float32)
    for c in range(N_CORES):
        b, qs = c // 4, (c % 4) * QLEN
        full[b, qs:qs + QLEN, :] = res.results[c]["out"]
    return full
